# revision 35
# baseline (speedup 1.0000x reference)
"""Trainium2 Bass kernel for nn_LoRALinear (DoRA-style LoRA linear).

Reference math (per problem):
    base = x @ W^T
    lora = sc * (x @ A^T) @ B^T          (sc = 2.0)
    w_eff = W + sc * (B @ A)
    s = magnitude / ||w_eff||_row         (row norm over in_dim)
    out = base + (s - 1) * base + s * lora
        = s * (base + lora)
        = x @ (s[:, None] * w_eff)^T

The whole op collapses to one dense matmul with a derived weight. The
derived weight is tiny (1024x1024) so it is computed on the host in fp32
and shipped per-core as bf16; the device does nothing but the big GEMM.

Sharding: data-parallel over batch*seq across 8 cores (4096 tokens each).

Layouts (host-prepared so the device never transposes):
  xt:  per core, token tile m (128 tokens) stored as [128 q, 8*128 (k,t)]
       with q = d_in within k-strip, t = token within tile. Each k-slice
       [:, k*128:(k+1)*128] is directly the stationary lhsT of a matmul.
  wT:  [d_in, d_out] bf16 as 8 k-strips [128, 1024]; each (k, h) slice
       is a moving rhs.
  out: [tokens, d_out] bf16, PSUM fp32 accumulate, converted on drain.

Schedule: the weight strips ride the ACT hardware-DGE queue while x and
out ride the SP queue, so the two input streams land in parallel right
after the fixed ~7.5us kernel preamble. The first four token tiles are
computed K-OUTER across all 8 PSUM banks: the PE consumes weight strip k
(for all 4 tiles x 2 halves) the moment it arrives, ramping its p-state
on real work with no large stall (large stalls reset the ramp and cost
~2x cycles for 3us). By the time the opening block finishes (~24us) all
weights are resident and the remaining 28 tiles stream at the PE peak of
~216ns per 512-row bf16 matmul.
"""

import os
import numpy as np
import ml_dtypes
from contextlib import ExitStack

import concourse.bass as bass
import concourse.mybir as mybir
import concourse.tile as tile
from concourse import bacc
from concourse.bass import ts
from concourse.bass_utils import run_bass_kernel_spmd

N_CORES = 8
B, S, D_IN, D_OUT, R = 4, 8192, 1024, 1024, 16
SCALING = 32.0 / 16.0
M_TOT = B * S                 # 32768 tokens
M_CORE = M_TOT // N_CORES     # 4096 tokens per core
P = 128
M_TILES = M_CORE // P         # 32
K_TILES = D_IN // P           # 8
NH = D_OUT // 512             # 2 n-halves of 512
M_BLOCK = 4                   # opening k-outer block width (uses 8 psum banks)
F32 = mybir.dt.float32
BF16 = mybir.dt.bfloat16
NP_BF16 = ml_dtypes.bfloat16


def _kernel_body(ctx: ExitStack, tc: "tile.TileContext", xt, wT, xw0_d, out):
    nc = tc.nc
    w_pool = ctx.enter_context(tc.tile_pool(name="w", bufs=1))
    x_pool = ctx.enter_context(tc.tile_pool(name="x", bufs=6))
    o_pool = ctx.enter_context(tc.tile_pool(name="o", bufs=6))
    ps_pool = ctx.enter_context(tc.tile_pool(name="ps", bufs=8, space="PSUM"))

    # the first matmul is gated on x0 AND strip 0: ship them as ONE
    # transfer (the SP queue is FIFO, so a fused first transfer completes
    # ~1.5us before two queued ones) and slice the tile afterwards
    xw0 = w_pool.tile([P, D_IN + D_OUT], BF16, tag="xw0", name="xw0")
    nc.sync.dma_start(xw0[:], xw0_d[:, :])

    # remaining weight strips on the ACT hardware-DGE queue, in parallel
    w_sb = [xw0[:, D_IN:]]
    for k in range(1, K_TILES):
        wk = w_pool.tile([P, D_OUT], BF16, tag=f"w{k}", name=f"w{k}")
        nc.scalar.dma_start(wk[:], wT[ts(k, P), :])
        w_sb.append(wk)

    # x tiles for the opening block follow on the SP queue
    x_sbs = {0: xw0[:, :D_IN]}
    for m in range(1, M_BLOCK):
        x_sbs[m] = x_pool.tile([P, D_IN], BF16, tag="xt", name=f"xt{m}")
        nc.sync.dma_start(x_sbs[m][:], xt[ts(m, P), :])

    # PE p-state warmup: the PE runs at ~2x cycle time until it has been
    # continuously busy for ~3us. 8 junk matmuls (no DMA dependency,
    # ~427ns each at the mid p-state) start as soon as the preamble ends
    # (~7.9us) and finish right as the first data lands (~11.3us), so
    # the real stream starts at the full 216ns cadence
    junk = w_pool.tile([P, 512], BF16, tag="junk", name="junk")
    nc.vector.memset(junk[:], 0.0)
    for j in range(8):
        psj = ps_pool.tile([P, 512], F32, tag="ps", name=f"warm{j}")
        nc.tensor.matmul(
            psj[:], lhsT=junk[:, :P], rhs=junk[:], start=True, stop=True
        )

    def drain_and_store(m, o_sb, ps, h, last=False):
        if last:
            # final drain + store are on the critical path: two separate
            # o tiles (same-tile writes from two engines would serialize)
            # drained on ACT and DVE, stored via both DGE queues
            o2 = o_pool.tile([P, 256], BF16, tag="o2", name="o_last")
            nc.scalar.copy(o2[:], ps[:, :256])
            nc.vector.tensor_copy(o_sb[:, 256:], ps[:, 256:])
            nc.scalar.dma_start(out[ts(m, P), ts(2 * h, 256)], o2[:])
            nc.sync.dma_start(out[ts(m, P), ts(2 * h + 1, 256)], o_sb[:, 256:])
            return
        if h == 0:
            nc.scalar.copy(o_sb[:], ps[:])
        else:
            nc.vector.tensor_copy(o_sb[:], ps[:])
        nc.sync.dma_start(out[ts(m, P), ts(h, 512)], o_sb[:])

    # opening block: k-outer over the first M_BLOCK tiles, all 8 psum
    # banks live, so strip k is fully consumed as soon as it lands
    ps_blk = [
        [ps_pool.tile([P, 512], F32, tag="ps", name=f"psb{m}_{h}") for h in range(NH)]
        for m in range(M_BLOCK)
    ]
    for k in range(K_TILES):
        for m in range(M_BLOCK):
            for h in range(NH):
                nc.tensor.matmul(
                    ps_blk[m][h][:],
                    lhsT=x_sbs[m][:, ts(k, P)] if m > 0 else xw0[:, ts(k, P)],
                    rhs=w_sb[k][:, ts(h, 512)],
                    start=(k == 0),
                    stop=(k == K_TILES - 1),
                )
    for m in range(M_BLOCK):
        for h in range(NH):
            o_sb = o_pool.tile([P, 512], BF16, tag="o")
            drain_and_store(m, o_sb, ps_blk[m][h], h)

    # steady state: per-tile, h-inner, 2 psum banks per tile
    for m in range(M_BLOCK, M_TILES):
        x_sb = x_pool.tile([P, D_IN], BF16, tag="xt", name=f"xt{m}")
        nc.sync.dma_start(x_sb[:], xt[ts(m, P), :])
        for h in range(NH):
            last = m == M_TILES - 1 and h == NH - 1
            if last:
                # the very last drain + 128KB store sit fully exposed
                # after the final matmul (exec ends ~2.5us after the
                # last out packet). Split this half into two 256-col
                # psum groups: group A's drain/store overlap group B's
                # matmuls, exposing only a 64KB tail
                for q in range(2):
                    psq = ps_pool.tile([P, 256], F32, tag="ps", name=f"ps_last{q}")
                    for k in range(K_TILES):
                        nc.tensor.matmul(
                            psq[:],
                            lhsT=x_sb[:, ts(k, P)],
                            rhs=w_sb[k][:, ts(2 * h + q, 256)],
                            start=(k == 0),
                            stop=(k == K_TILES - 1),
                        )
                    oq = o_pool.tile([P, 256], BF16, tag="o2", name=f"o_last{q}")
                    if q == 0:
                        nc.scalar.copy(oq[:], psq[:])
                        nc.scalar.dma_start(out[ts(m, P), ts(2 * h + q, 256)], oq[:])
                    else:
                        nc.vector.tensor_copy(oq[:], psq[:])
                        nc.sync.dma_start(out[ts(m, P), ts(2 * h + q, 256)], oq[:])
                continue
            ps = ps_pool.tile([P, 512], F32, tag="ps")
            for k in range(K_TILES):
                nc.tensor.matmul(
                    ps[:],
                    lhsT=x_sb[:, ts(k, P)],
                    rhs=w_sb[k][:, ts(h, 512)],
                    start=(k == 0),
                    stop=(k == K_TILES - 1),
                )
            o_sb = o_pool.tile([P, 512], BF16, tag="o")
            drain_and_store(m, o_sb, ps, h, False)


def build_nc() -> "bass.Bass":
    nc = bacc.Bacc(
        "TRN2",
        target_bir_lowering=False,
        debug=False,
        num_devices=N_CORES,
    )
    xt = nc.dram_tensor("xt", [M_CORE, D_IN], BF16, kind="ExternalInput").ap()
    wT = nc.dram_tensor("wT", [D_IN, D_OUT], BF16, kind="ExternalInput").ap()
    xw0_d = nc.dram_tensor(
        "xw0", [P, D_IN + D_OUT], BF16, kind="ExternalInput"
    ).ap()
    out = nc.dram_tensor("out", [M_CORE, D_OUT], BF16, kind="ExternalOutput").ap()

    with tile.TileContext(nc) as tc, ExitStack() as ctx:
        _kernel_body(ctx, tc, xt, wT, xw0_d, out)
    nc.compile()
    return nc


_NC_CACHE: list = []


def get_nc() -> "bass.Bass":
    if not _NC_CACHE:
        _NC_CACHE.append(build_nc())
    return _NC_CACHE[0]


def make_in_maps(x, weight, a_w, b_w, magnitude):
    # derived DoRA weight, fully on host (tiny: 1024x1024)
    wf = weight.astype(np.float32, copy=False)
    w_eff = wf + SCALING * (b_w.astype(np.float32) @ a_w.astype(np.float32))
    norm = np.sqrt((w_eff.astype(np.float64) ** 2).sum(axis=1))
    s = (magnitude.reshape(-1) / norm).astype(np.float32)
    wT = np.ascontiguousarray((s[:, None] * w_eff).T).astype(NP_BF16)

    # x: per-core PE-ready tiles; tile m holds [q, k*128 + t] =
    # x[m*128 + t, k*128 + q] so each k-slice is a matmul lhsT
    xf = x.reshape(M_TOT, D_IN).astype(np.float32, copy=False)
    in_maps = []
    for c in range(N_CORES):
        xc = xf[c * M_CORE : (c + 1) * M_CORE]
        ht = xc.reshape(M_TILES, P, K_TILES, P).transpose(0, 3, 2, 1)
        xt = np.ascontiguousarray(ht.astype(NP_BF16)).reshape(M_CORE, D_IN)
        xw0 = np.ascontiguousarray(np.concatenate([xt[:P], wT[:P]], axis=1))
        in_maps.append({"xt": xt, "wT": wT, "xw0": xw0})
    return in_maps


def kernel(x, weight, a_w, b_w, magnitude):
    nc = get_nc()
    in_maps = make_in_maps(x, weight, a_w, b_w, magnitude)
    trace = os.environ.get("KERNEL_TRACE", "0") == "1"
    res = run_bass_kernel_spmd(nc, in_maps, list(range(N_CORES)), trace=trace)
    if trace:
        kernel.last_result = res
    outs = [res.results[i]["out"].astype(np.float32) for i in range(N_CORES)]
    return np.concatenate(outs, axis=0).reshape(B, S, D_OUT)


# revision 38
# speedup vs baseline: 1.1844x; 1.1844x over previous
"""Trainium2 Bass kernel for nn_LoRALinear (DoRA-style LoRA linear).

Reference math (per problem):
    base = x @ W^T
    lora = sc * (x @ A^T) @ B^T          (sc = 2.0)
    w_eff = W + sc * (B @ A)
    s = magnitude / ||w_eff||_row         (row norm over in_dim)
    out = base + (s - 1) * base + s * lora
        = s * (base + lora)
        = x @ (s[:, None] * w_eff)^T

The whole op collapses to one dense matmul with a derived weight. The
derived weight is tiny (1024x1024) so it is computed on the host in fp32
and shipped per-core as bf16; the device does nothing but the big GEMM.

Sharding: data-parallel over batch*seq across 8 cores (4096 tokens each).

Layouts (host-prepared so the device never transposes):
  xt:  per core, token tile m (128 tokens) stored as [128 q, 8*128 (k,t)]
       with q = d_in within k-strip, t = token within tile. Each k-slice
       [:, k*128:(k+1)*128] is directly the stationary lhsT of a matmul.
  wT:  [d_in, d_out] bf16 as 8 k-strips [128, 1024]; each (k, h) slice
       is a moving rhs.
  out: [tokens, d_out] bf16, PSUM fp32 accumulate, converted on drain.

Schedule: the weight strips ride the ACT hardware-DGE queue while x and
out ride the SP queue, so the two input streams land in parallel right
after the fixed ~7.5us kernel preamble. The first four token tiles are
computed K-OUTER across all 8 PSUM banks: the PE consumes weight strip k
(for all 4 tiles x 2 halves) the moment it arrives, ramping its p-state
on real work with no large stall (large stalls reset the ramp and cost
~2x cycles for 3us). By the time the opening block finishes (~24us) all
weights are resident and the remaining 28 tiles stream at the PE peak of
~216ns per 512-row bf16 matmul.
"""

import os
import numpy as np
import ml_dtypes
from contextlib import ExitStack

import concourse.bass as bass
import concourse.mybir as mybir
import concourse.tile as tile
from concourse import bacc
from concourse.bass import ts
from concourse.bass_utils import run_bass_kernel_spmd

N_CORES = 8
B, S, D_IN, D_OUT, R = 4, 8192, 1024, 1024, 16
SCALING = 32.0 / 16.0
M_TOT = B * S                 # 32768 tokens
M_CORE = M_TOT // N_CORES     # 4096 tokens per core
P = 128
M_TILES = M_CORE // P         # 32
K_TILES = D_IN // P           # 8
NH = D_OUT // 512             # 2 n-halves of 512
M_BLOCK = 4                   # opening k-outer block width (uses 8 psum banks)
F32 = mybir.dt.float32
BF16 = mybir.dt.bfloat16
NP_BF16 = ml_dtypes.bfloat16


def _kernel_body(ctx: ExitStack, tc: "tile.TileContext", xt, wT, xw0_d, out):
    nc = tc.nc
    w_pool = ctx.enter_context(tc.tile_pool(name="w", bufs=1))
    x_pool = ctx.enter_context(tc.tile_pool(name="x", bufs=6))
    o_pool = ctx.enter_context(tc.tile_pool(name="o", bufs=6))
    ps_pool = ctx.enter_context(tc.tile_pool(name="ps", bufs=8, space="PSUM"))

    # the first matmul is gated on x0 AND strip 0: ship them as ONE
    # transfer (the SP queue is FIFO, so a fused first transfer completes
    # ~1.5us before two queued ones) and slice the tile afterwards
    xw0 = w_pool.tile([P, D_IN + D_OUT], BF16, tag="xw0", name="xw0")
    nc.sync.dma_start(xw0[:], xw0_d[:, :])

    # remaining weight strips on the ACT hardware-DGE queue, in parallel
    w_sb = [xw0[:, D_IN:]]
    for k in range(1, K_TILES):
        wk = w_pool.tile([P, D_OUT], BF16, tag=f"w{k}", name=f"w{k}")
        nc.scalar.dma_start(wk[:], wT[ts(k, P), :])
        w_sb.append(wk)

    # x tiles for the opening block follow on the SP queue
    x_sbs = {0: xw0[:, :D_IN]}
    for m in range(1, M_BLOCK):
        x_sbs[m] = x_pool.tile([P, D_IN], BF16, tag="xt", name=f"xt{m}")
        nc.sync.dma_start(x_sbs[m][:], xt[ts(m, P), :])

    # PE p-state warmup: the PE runs at ~2x cycle time until it has been
    # continuously busy for ~3us. 8 junk matmuls (no DMA dependency,
    # ~427ns each at the mid p-state) start as soon as the preamble ends
    # (~7.9us) and finish right as the first data lands (~11.3us), so
    # the real stream starts at the full 216ns cadence
    junk = w_pool.tile([P, 512], BF16, tag="junk", name="junk")
    nc.vector.memset(junk[:], 0.0)
    for j in range(8):
        psj = ps_pool.tile([P, 512], F32, tag="ps", name=f"warm{j}")
        nc.tensor.matmul(
            psj[:], lhsT=junk[:, :P], rhs=junk[:], start=True, stop=True
        )

    def drain_and_store(m, o_sb, ps, h, last=False):
        if last:
            # final drain + store are on the critical path: two separate
            # o tiles (same-tile writes from two engines would serialize)
            # drained on ACT and DVE, stored via both DGE queues
            o2 = o_pool.tile([P, 256], BF16, tag="o2", name="o_last")
            nc.scalar.copy(o2[:], ps[:, :256])
            nc.vector.tensor_copy(o_sb[:, 256:], ps[:, 256:])
            nc.scalar.dma_start(out[ts(m, P), ts(2 * h, 256)], o2[:])
            nc.sync.dma_start(out[ts(m, P), ts(2 * h + 1, 256)], o_sb[:, 256:])
            return
        if h == 0:
            nc.scalar.copy(o_sb[:], ps[:])
        else:
            nc.vector.tensor_copy(o_sb[:], ps[:])
        nc.sync.dma_start(out[ts(m, P), ts(h, 512)], o_sb[:])

    # opening block: k-outer over the first M_BLOCK tiles, all 8 psum
    # banks live, so strip k is fully consumed as soon as it lands
    ps_blk = [
        [ps_pool.tile([P, 512], F32, tag="ps", name=f"psb{m}_{h}") for h in range(NH)]
        for m in range(M_BLOCK)
    ]
    for k in range(K_TILES):
        for m in range(M_BLOCK):
            for h in range(NH):
                nc.tensor.matmul(
                    ps_blk[m][h][:],
                    lhsT=x_sbs[m][:, ts(k, P)] if m > 0 else xw0[:, ts(k, P)],
                    rhs=w_sb[k][:, ts(h, 512)],
                    start=(k == 0),
                    stop=(k == K_TILES - 1),
                )
    for m in range(M_BLOCK):
        for h in range(NH):
            o_sb = o_pool.tile([P, 512], BF16, tag="o")
            drain_and_store(m, o_sb, ps_blk[m][h], h)

    # steady state: per-tile, h-inner, 2 psum banks per tile
    for m in range(M_BLOCK, M_TILES):
        x_sb = x_pool.tile([P, D_IN], BF16, tag="xt", name=f"xt{m}")
        nc.sync.dma_start(x_sb[:], xt[ts(m, P), :])
        for h in range(NH):
            ps = ps_pool.tile([P, 512], F32, tag="ps")
            for k in range(K_TILES):
                nc.tensor.matmul(
                    ps[:],
                    lhsT=x_sb[:, ts(k, P)],
                    rhs=w_sb[k][:, ts(h, 512)],
                    start=(k == 0),
                    stop=(k == K_TILES - 1),
                )
            o_sb = o_pool.tile([P, 512], BF16, tag="o")
            last = m == M_TILES - 1 and h == NH - 1
            drain_and_store(m, o_sb, ps, h, last)


def build_nc() -> "bass.Bass":
    nc = bacc.Bacc(
        "TRN2",
        target_bir_lowering=False,
        debug=False,
        num_devices=N_CORES,
    )
    xt = nc.dram_tensor("xt", [M_CORE, D_IN], BF16, kind="ExternalInput").ap()
    wT = nc.dram_tensor("wT", [D_IN, D_OUT], BF16, kind="ExternalInput").ap()
    xw0_d = nc.dram_tensor(
        "xw0", [P, D_IN + D_OUT], BF16, kind="ExternalInput"
    ).ap()
    out = nc.dram_tensor("out", [M_CORE, D_OUT], BF16, kind="ExternalOutput").ap()

    with tile.TileContext(nc) as tc, ExitStack() as ctx:
        _kernel_body(ctx, tc, xt, wT, xw0_d, out)
    nc.compile()
    return nc


_NC_CACHE: list = []


def get_nc() -> "bass.Bass":
    if not _NC_CACHE:
        _NC_CACHE.append(build_nc())
    return _NC_CACHE[0]


def make_in_maps(x, weight, a_w, b_w, magnitude):
    # derived DoRA weight, fully on host (tiny: 1024x1024)
    wf = weight.astype(np.float32, copy=False)
    w_eff = wf + SCALING * (b_w.astype(np.float32) @ a_w.astype(np.float32))
    norm = np.sqrt((w_eff.astype(np.float64) ** 2).sum(axis=1))
    s = (magnitude.reshape(-1) / norm).astype(np.float32)
    wT = np.ascontiguousarray((s[:, None] * w_eff).T).astype(NP_BF16)

    # x: per-core PE-ready tiles; tile m holds [q, k*128 + t] =
    # x[m*128 + t, k*128 + q] so each k-slice is a matmul lhsT
    xf = x.reshape(M_TOT, D_IN).astype(np.float32, copy=False)
    in_maps = []
    for c in range(N_CORES):
        xc = xf[c * M_CORE : (c + 1) * M_CORE]
        ht = xc.reshape(M_TILES, P, K_TILES, P).transpose(0, 3, 2, 1)
        xt = np.ascontiguousarray(ht.astype(NP_BF16)).reshape(M_CORE, D_IN)
        xw0 = np.ascontiguousarray(np.concatenate([xt[:P], wT[:P]], axis=1))
        in_maps.append({"xt": xt, "wT": wT, "xw0": xw0})
    return in_maps


def kernel(x, weight, a_w, b_w, magnitude):
    nc = get_nc()
    in_maps = make_in_maps(x, weight, a_w, b_w, magnitude)
    trace = os.environ.get("KERNEL_TRACE", "0") == "1"
    res = run_bass_kernel_spmd(nc, in_maps, list(range(N_CORES)), trace=trace)
    if trace:
        kernel.last_result = res
    outs = [res.results[i]["out"].astype(np.float32) for i in range(N_CORES)]
    return np.concatenate(outs, axis=0).reshape(B, S, D_OUT)


# revision 39
# speedup vs baseline: 1.1846x; 1.0002x over previous
"""Trainium2 Bass kernel for nn_LoRALinear (DoRA-style LoRA linear).

Reference math (per problem):
    base = x @ W^T
    lora = sc * (x @ A^T) @ B^T          (sc = 2.0)
    w_eff = W + sc * (B @ A)
    s = magnitude / ||w_eff||_row         (row norm over in_dim)
    out = base + (s - 1) * base + s * lora
        = s * (base + lora)
        = x @ (s[:, None] * w_eff)^T

The whole op collapses to one dense matmul with a derived weight. The
derived weight is tiny (1024x1024) so it is computed on the host in fp32
and shipped per-core as bf16; the device does nothing but the big GEMM.

Sharding: data-parallel over batch*seq across 8 cores (4096 tokens each).

Layouts (host-prepared so the device never transposes):
  xt:  per core, token tile m (128 tokens) stored as [128 q, 8*128 (k,t)]
       with q = d_in within k-strip, t = token within tile. Each k-slice
       [:, k*128:(k+1)*128] is directly the stationary lhsT of a matmul.
  wT:  [d_in, d_out] bf16 as 8 k-strips [128, 1024]; each (k, h) slice
       is a moving rhs.
  out: [tokens, d_out] bf16, PSUM fp32 accumulate, converted on drain.

Schedule: the weight strips ride the ACT hardware-DGE queue while x and
out ride the SP queue, so the two input streams land in parallel right
after the fixed ~7.5us kernel preamble. The first four token tiles are
computed K-OUTER across all 8 PSUM banks: the PE consumes weight strip k
(for all 4 tiles x 2 halves) the moment it arrives, ramping its p-state
on real work with no large stall (large stalls reset the ramp and cost
~2x cycles for 3us). By the time the opening block finishes (~24us) all
weights are resident and the remaining 28 tiles stream at the PE peak of
~216ns per 512-row bf16 matmul.
"""

import os
import numpy as np
import ml_dtypes
from contextlib import ExitStack

import concourse.bass as bass
import concourse.mybir as mybir
import concourse.tile as tile
from concourse import bacc
from concourse.bass import ts
from concourse.bass_utils import run_bass_kernel_spmd

N_CORES = 8
B, S, D_IN, D_OUT, R = 4, 8192, 1024, 1024, 16
SCALING = 32.0 / 16.0
M_TOT = B * S                 # 32768 tokens
M_CORE = M_TOT // N_CORES     # 4096 tokens per core
P = 128
M_TILES = M_CORE // P         # 32
K_TILES = D_IN // P           # 8
NH = D_OUT // 512             # 2 n-halves of 512
M_BLOCK = 4                   # opening k-outer block width (uses 8 psum banks)
F32 = mybir.dt.float32
BF16 = mybir.dt.bfloat16
NP_BF16 = ml_dtypes.bfloat16


def _kernel_body(ctx: ExitStack, tc: "tile.TileContext", xt, wT, xw0_d, out):
    nc = tc.nc
    w_pool = ctx.enter_context(tc.tile_pool(name="w", bufs=1))
    x_pool = ctx.enter_context(tc.tile_pool(name="x", bufs=6))
    o_pool = ctx.enter_context(tc.tile_pool(name="o", bufs=6))
    ps_pool = ctx.enter_context(tc.tile_pool(name="ps", bufs=8, space="PSUM"))

    # the first matmul is gated on x0 AND strip 0: ship them as ONE
    # transfer (the SP queue is FIFO, so a fused first transfer completes
    # ~1.5us before two queued ones) and slice the tile afterwards
    xw0 = w_pool.tile([P, D_IN + D_OUT], BF16, tag="xw0", name="xw0")
    nc.sync.dma_start(xw0[:], xw0_d[:, :])

    # remaining weight strips on the ACT hardware-DGE queue, in parallel
    w_sb = [xw0[:, D_IN:]]
    for k in range(1, K_TILES):
        wk = w_pool.tile([P, D_OUT], BF16, tag=f"w{k}", name=f"w{k}")
        nc.scalar.dma_start(wk[:], wT[ts(k, P), :])
        w_sb.append(wk)

    # x tiles for the opening block follow on the SP queue
    x_sbs = {0: xw0[:, :D_IN]}
    for m in range(1, M_BLOCK):
        x_sbs[m] = x_pool.tile([P, D_IN], BF16, tag="xt", name=f"xt{m}")
        nc.sync.dma_start(x_sbs[m][:], xt[ts(m, P), :])

    # PE p-state warmup: the PE runs at ~2x cycle time until it has been
    # continuously busy for ~3us. 8 junk matmuls (no DMA dependency,
    # ~427ns each at the mid p-state) start as soon as the preamble ends
    # (~7.9us) and finish right as the first data lands (~11.3us), so
    # the real stream starts at the full 216ns cadence
    junk = w_pool.tile([P, 512], BF16, tag="junk", name="junk")
    nc.vector.memset(junk[:], 0.0)
    for j in range(8):
        psj = ps_pool.tile([P, 512], F32, tag="ps", name=f"warm{j}")
        nc.tensor.matmul(
            psj[:], lhsT=junk[:, :P], rhs=junk[:], start=True, stop=True
        )

    def drain_and_store(m, o_sb, ps, h, last=False):
        if last:
            # final drain + store are on the critical path: two separate
            # o tiles (same-tile writes from two engines would serialize)
            # drained on ACT and DVE, stored via both DGE queues
            o2 = o_pool.tile([P, 256], BF16, tag="o2", name="o_last")
            nc.scalar.copy(o2[:], ps[:, :256])
            nc.vector.tensor_copy(o_sb[:, 256:], ps[:, 256:])
            nc.scalar.dma_start(out[ts(m, P), ts(2 * h, 256)], o2[:])
            nc.sync.dma_start(out[ts(m, P), ts(2 * h + 1, 256)], o_sb[:, 256:])
            return
        if h == 0:
            nc.scalar.copy(o_sb[:], ps[:])
        else:
            nc.vector.tensor_copy(o_sb[:], ps[:])
        nc.sync.dma_start(out[ts(m, P), ts(h, 512)], o_sb[:])

    # opening block: k-outer over the first M_BLOCK tiles, all 8 psum
    # banks live, so strip k is fully consumed as soon as it lands
    ps_blk = [
        [ps_pool.tile([P, 512], F32, tag="ps", name=f"psb{m}_{h}") for h in range(NH)]
        for m in range(M_BLOCK)
    ]
    for k in range(K_TILES):
        for m in range(M_BLOCK):
            for h in range(NH):
                nc.tensor.matmul(
                    ps_blk[m][h][:],
                    lhsT=x_sbs[m][:, ts(k, P)] if m > 0 else xw0[:, ts(k, P)],
                    rhs=w_sb[k][:, ts(h, 512)],
                    start=(k == 0),
                    stop=(k == K_TILES - 1),
                )
    for m in range(M_BLOCK):
        for h in range(NH):
            o_sb = o_pool.tile([P, 512], BF16, tag="o")
            drain_and_store(m, o_sb, ps_blk[m][h], h)

    # steady state: per-tile, h-inner, 2 psum banks per tile
    for m in range(M_BLOCK, M_TILES):
        x_sb = x_pool.tile([P, D_IN], BF16, tag="xt", name=f"xt{m}")
        nc.sync.dma_start(x_sb[:], xt[ts(m, P), :])
        for h in range(NH):
            ps = ps_pool.tile([P, 512], F32, tag="ps")
            for k in range(K_TILES):
                nc.tensor.matmul(
                    ps[:],
                    lhsT=x_sb[:, ts(k, P)],
                    rhs=w_sb[k][:, ts(h, 512)],
                    start=(k == 0),
                    stop=(k == K_TILES - 1),
                )
            o_sb = o_pool.tile([P, 512], BF16, tag="o")
            last = m == M_TILES - 1 and h == NH - 1
            drain_and_store(m, o_sb, ps, h, last)


def build_nc() -> "bass.Bass":
    nc = bacc.Bacc(
        "TRN2",
        target_bir_lowering=False,
        debug=False,
        num_devices=N_CORES,
    )
    xt = nc.dram_tensor("xt", [M_CORE, D_IN], BF16, kind="ExternalInput").ap()
    wT = nc.dram_tensor("wT", [D_IN, D_OUT], BF16, kind="ExternalInput").ap()
    xw0_d = nc.dram_tensor(
        "xw0", [P, D_IN + D_OUT], BF16, kind="ExternalInput"
    ).ap()
    out = nc.dram_tensor("out", [M_CORE, D_OUT], BF16, kind="ExternalOutput").ap()

    with tile.TileContext(nc, pool_alloc_mode="queue") as tc, ExitStack() as ctx:
        _kernel_body(ctx, tc, xt, wT, xw0_d, out)
    nc.compile()
    return nc


_NC_CACHE: list = []


def get_nc() -> "bass.Bass":
    if not _NC_CACHE:
        _NC_CACHE.append(build_nc())
    return _NC_CACHE[0]


def make_in_maps(x, weight, a_w, b_w, magnitude):
    # derived DoRA weight, fully on host (tiny: 1024x1024)
    wf = weight.astype(np.float32, copy=False)
    w_eff = wf + SCALING * (b_w.astype(np.float32) @ a_w.astype(np.float32))
    norm = np.sqrt((w_eff.astype(np.float64) ** 2).sum(axis=1))
    s = (magnitude.reshape(-1) / norm).astype(np.float32)
    wT = np.ascontiguousarray((s[:, None] * w_eff).T).astype(NP_BF16)

    # x: per-core PE-ready tiles; tile m holds [q, k*128 + t] =
    # x[m*128 + t, k*128 + q] so each k-slice is a matmul lhsT
    xf = x.reshape(M_TOT, D_IN).astype(np.float32, copy=False)
    in_maps = []
    for c in range(N_CORES):
        xc = xf[c * M_CORE : (c + 1) * M_CORE]
        ht = xc.reshape(M_TILES, P, K_TILES, P).transpose(0, 3, 2, 1)
        xt = np.ascontiguousarray(ht.astype(NP_BF16)).reshape(M_CORE, D_IN)
        xw0 = np.ascontiguousarray(np.concatenate([xt[:P], wT[:P]], axis=1))
        in_maps.append({"xt": xt, "wT": wT, "xw0": xw0})
    return in_maps


def kernel(x, weight, a_w, b_w, magnitude):
    nc = get_nc()
    in_maps = make_in_maps(x, weight, a_w, b_w, magnitude)
    trace = os.environ.get("KERNEL_TRACE", "0") == "1"
    res = run_bass_kernel_spmd(nc, in_maps, list(range(N_CORES)), trace=trace)
    if trace:
        kernel.last_result = res
    outs = [res.results[i]["out"].astype(np.float32) for i in range(N_CORES)]
    return np.concatenate(outs, axis=0).reshape(B, S, D_OUT)
